# revision 1
# baseline (speedup 1.0000x reference)
"""Trainium2 Bass kernel for LocalizationLoss (box MSE + cross-entropy, batch mean).

Input : output [262144, 1004] f32  (cols 0:4 = box pred cx,cy,w,h; cols 4:1004 = logits)
        target [262144, 5]    f32  (xmin,ymin,xmax,ymax,class_id)
Output: scalar f32 = mean_b( mean_4((box_pred-box_true)^2) + CE(logits, class) )

Strategy (pure data parallel over 8 cores, 32768 rows each):
  - rows mapped p-major: partition p owns rows p*256..p*256+255 of its shard
  - stream 32 groups of 8 row-tiles [128, 8, 1004]; one big DMA per group
  - ScalarE: exp over logits with fused row-sum (accum_out -> PSUM) = sumexp
  - VectorE: picked logit via one scalar_tensor_tensor per tile:
        out = (iota is_equal class_p) * logits ; accum_out = logits[p, class_p]
    (iota is a [128,1000] constant input 0..999 per row; class_p is the f32
     class id as a per-partition scalar AP)
  - GpSimdE: box-error terms per group as doubled differences (TensorTensor
    only); ScalarE Square(scale=0.5) with accum_out sums all 4 components
  - epilogue: logZ = Ln(sumexp) with fused sum; CE_sum = logZ_sum - picked_sum
  - each core returns [128,1] per-partition partial sums; host adds and /B

This container's walrus build accepts at most ONE sync-wait per instruction,
while the Tile scheduler attaches several. `_split_multiwait_bir` rewrites the
serialized BIR to hoist extra waits onto single-wait NoOp carriers, and is
installed as a wrapper around compile_bir_kernel at import time. The same
walrus also cannot lower the custom-DVE ISA ops (tensor_mask_reduce etc.) or
Pool-engine TensorScalarPtr, so only standard opcodes are used.
"""

import json as _json

import numpy as np

import concourse.bass as bass
import concourse.tile as tile
from concourse import mybir
import concourse.bass_utils as _bass_utils
import concourse.bass2jax as _bass2jax
from concourse.bass_utils import run_bass_kernel_spmd

P = 128
B = 262144
C = 1004
NCLS = 1000
NCORES = 8
R = B // NCORES       # 32768 rows per core
T = R // P            # 256 row-tiles per core (rows per partition)
G = 8                 # row-tiles per group
NG = T // G           # 32 groups

F32 = mybir.dt.float32
ALU = mybir.AluOpType
ACTF = mybir.ActivationFunctionType


# --------------------------------------------------------------------------
# BIR post-pass: this image's walrus supports only one sync-wait per
# instruction; split extras onto NoOp carriers placed just before.
# --------------------------------------------------------------------------
def _split_multiwait_bir(bir_json: bytes) -> bytes:
    d = _json.loads(bir_json)
    changed = False
    for fn in d.get("functions", []):
        for blk in fn.get("blocks", []):
            insts = blk.get("instructions", [])
            out = []
            for ins in insts:
                si = ins.get("sync_info") or {}
                waits = si.get("on_wait") or []
                if len(waits) > 1:
                    changed = True
                    for i, w in enumerate(waits[:-1]):
                        out.append(
                            {
                                "debug": ins.get("debug", 0),
                                "engine": ins["engine"],
                                "ins": [],
                                "name": f"{ins['name']}-wsplit{i}",
                                "opcode": "NoOp",
                                "outs": [],
                                "sync_info": {"on_update": [], "on_wait": [w]},
                            }
                        )
                    ins["sync_info"]["on_wait"] = [waits[-1]]
                out.append(ins)
            blk["instructions"] = out
    if not changed:
        return bir_json
    return _json.dumps(d).encode()


_orig_compile_bir_kernel = _bass_utils.compile_bir_kernel


def _compile_bir_kernel_fixed(bir_json, tmpdir, neff_name="file.neff"):
    if isinstance(bir_json, str):
        bir_json = bir_json.encode()
    return _orig_compile_bir_kernel(_split_multiwait_bir(bir_json), tmpdir, neff_name)


if _bass_utils.compile_bir_kernel is not _compile_bir_kernel_fixed:
    _bass_utils.compile_bir_kernel = _compile_bir_kernel_fixed
    _bass2jax.compile_bir_kernel = _compile_bir_kernel_fixed


# --------------------------------------------------------------------------
# kernel build
# --------------------------------------------------------------------------
def build():
    nc = bass.Bass()
    x = nc.dram_tensor("x", [R, C], F32, kind="ExternalInput")
    t = nc.dram_tensor("t", [R, 5], F32, kind="ExternalInput")
    iota_in = nc.dram_tensor("iota", [P, NCLS], F32, kind="ExternalInput")
    out = nc.dram_tensor("partial", [P, 1], F32, kind="ExternalOutput")

    xv = x[:].rearrange("(p n) c -> p n c", p=P)   # [128, 256, 1004]
    tv = t[:].rearrange("(p n) f -> p n f", p=P)   # [128, 256, 5]

    with tile.TileContext(nc) as tc:
        with (
            tc.tile_pool(name="data", bufs=4) as data_pool,
            tc.tile_pool(name="scr", bufs=2) as scr_pool,
            tc.tile_pool(name="acc", bufs=1) as acc_pool,
        ):
            iota_t = acc_pool.tile([P, NCLS], F32)
            nc.sync.dma_start(out=iota_t, in_=iota_in[:])
            # whole per-core target resident: [128, 256, 5] = 5 KiB/partition,
            # one DMA with contiguous 5120B per-partition chunks
            tgt = acc_pool.tile([P, T, 5], F32)
            nc.sync.dma_start(out=tgt, in_=tv)

            # variable group sizes: small head groups shrink the pipeline
            # fill (compute starts after ~1MB instead of ~4MB), small tail
            # groups shrink the end-of-run compute drain
            group_sizes = [2, 2, 4] + [8] * 30 + [4, 2, 2]
            assert sum(group_sizes) == T
            n_groups = len(group_sizes)
            # tiles whose sumexp goes ACT-exp + DVE-reduce (engine balance)
            dve_sumexp_groups = {0, 6, 12, 18, 24, 30}

            sumexp_all = acc_pool.tile([P, T], F32)      # per-row sum(exp(logits))
            loc_all = acc_pool.tile([P, n_groups], F32)  # per-group sq-err sums
            picked_all = acc_pool.tile([P, T], F32)      # per-row logits[class]

            t0 = 0
            for grp, gs in enumerate(group_sizes):
                data = data_pool.tile([P, gs, C], F32, tag="data")
                nc.sync.dma_start(out=data, in_=xv[:, t0 : t0 + gs, :])

                # box-error terms as doubled differences (GpSimd TensorTensor
                # on [128, G, 2] views), then squared on GpSimd and summed by
                # one small VectorE reduce:
                #   e_cx_cy = (t01 + t23) - 2*bp01      -> (0.5*e)^2 = err^2
                #   e_wh    = 2*((t23 - t01) - bp23)    -> (0.5*e)^2 = err^2
                e4 = scr_pool.tile([P, 2, gs, 2], F32, tag="e4")
                u2 = scr_pool.tile([P, gs, 2], F32, tag="u2")
                t01 = tgt[:, t0 : t0 + gs, 0:2]
                t23 = tgt[:, t0 : t0 + gs, 2:4]
                bp01 = data[:, :, 0:2]
                bp23 = data[:, :, 2:4]
                nc.gpsimd.tensor_add(u2, t01, t23)
                nc.gpsimd.tensor_sub(u2, u2, bp01)
                nc.gpsimd.tensor_sub(e4[:, 0, :, :], u2, bp01)
                nc.gpsimd.tensor_sub(u2, t23, t01)
                nc.gpsimd.tensor_sub(u2, u2, bp23)
                nc.gpsimd.tensor_add(e4[:, 1, :, :], u2, u2)
                nc.gpsimd.tensor_mul(e4, e4, e4)
                nc.vector.tensor_reduce(
                    out=loc_all[:, grp : grp + 1], in_=e4,
                    axis=mybir.AxisListType.XYZ, op=ALU.add,
                )

                for g in range(gs):
                    tt = t0 + g
                    exp_scr = scr_pool.tile([P, NCLS], F32, tag="exp_scr")
                    # ScalarE is the busiest engine; for a slice of tiles do
                    # exp without the accumulator (saves the ~280ns
                    # READ_ACCUMULATOR per tile) and let VectorE reduce.
                    if grp in dve_sumexp_groups and g == 0:
                        nc.scalar.activation(
                            out=exp_scr, in_=data[:, g, 4:C], func=ACTF.Exp
                        )
                        nc.vector.tensor_reduce(
                            out=sumexp_all[:, tt : tt + 1],
                            in_=exp_scr,
                            axis=mybir.AxisListType.X,
                            op=ALU.add,
                        )
                    else:
                        nc.scalar.activation(
                            out=exp_scr,
                            in_=data[:, g, 4:C],
                            func=ACTF.Exp,
                            accum_out=sumexp_all[:, tt : tt + 1],
                        )
                    pick_scr = scr_pool.tile([P, NCLS], F32, tag="pick_scr")
                    nc.vector.scalar_tensor_tensor(
                        pick_scr,
                        iota_t,
                        tgt[:, tt, 4:5],
                        data[:, g, 4:C],
                        ALU.is_equal,
                        ALU.mult,
                        accum_out=picked_all[:, tt : tt + 1],
                    )
                t0 += gs

            # ---- epilogue ----
            logz_scr = acc_pool.tile([P, T], F32)
            logz_sum = acc_pool.tile([P, 1], F32)
            nc.scalar.activation(
                out=logz_scr, in_=sumexp_all, func=ACTF.Ln, accum_out=logz_sum
            )
            pick_sum = acc_pool.tile([P, 1], F32)
            nc.vector.tensor_reduce(
                out=pick_sum, in_=picked_all, axis=mybir.AxisListType.X, op=ALU.add
            )
            loc_sum = acc_pool.tile([P, 1], F32)
            nc.vector.tensor_reduce(
                out=loc_sum, in_=loc_all, axis=mybir.AxisListType.X, op=ALU.add
            )
            s = acc_pool.tile([P, 1], F32)
            # loc_all holds (2*err)^2 sums -> mean over 4 comps with the
            # doubling correction is 0.25 * 0.25 = 0.0625
            nc.vector.scalar_tensor_tensor(
                s, loc_sum, 0.0625, logz_sum, ALU.mult, ALU.add
            )
            nc.vector.tensor_sub(s, s, pick_sum)
            nc.sync.dma_start(out=out[:], in_=s)

    return nc


_IOTA = np.ascontiguousarray(
    np.broadcast_to(np.arange(NCLS, dtype=np.float32), (P, NCLS))
)


def _run(output, target, **spmd_kwargs):
    output = np.ascontiguousarray(np.asarray(output, dtype=np.float32))
    target = np.ascontiguousarray(np.asarray(target, dtype=np.float32))
    assert output.shape == (B, C), output.shape
    assert target.shape == (B, 5), target.shape
    nc = build()
    in_maps = [
        {
            "x": output[i * R : (i + 1) * R],
            "t": target[i * R : (i + 1) * R],
            "iota": _IOTA,
        }
        for i in range(NCORES)
    ]
    res = run_bass_kernel_spmd(nc, in_maps, core_ids=list(range(NCORES)), **spmd_kwargs)
    total = 0.0
    for r in res.results:
        total += r["partial"].astype(np.float64).sum()
    return np.float32(total / B), res


def kernel(output, target):
    val, _ = _run(output, target)
    return np.asarray(val, dtype=np.float32)


def kernel_profiled(output, target, **kw):
    """Returns (scalar, BassKernelResults) with trace for perf analysis."""
    return _run(output, target, trace=True, **kw)



# revision 2
# speedup vs baseline: 1.0179x; 1.0179x over previous
"""Trainium2 Bass kernel v3 for LocalizationLoss (box MSE + cross-entropy, batch mean).

Input : output [262144, 1004] f32  (cols 0:4 = box pred cx,cy,w,h; cols 4:1004 = logits)
        target [262144, 5]    f32  (xmin,ymin,xmax,ymax,class_id)
Output: scalar f32 = mean_b( mean_4((box_pred-box_true)^2) + CE(logits, class) )

v3 = v2 + mixed DMA routing. Trace facts driving the design:
  - SWDGE cast DMA (f32 HBM -> fp16 SBUF) runs most DMA engines at ~0.99us
    per 32KB descriptor, but engine 15 at ~1.18us (+19%, known SWDGE
    straggler) -> pure-fp16 floor is engine15 = ~383us.
  - HWDGE f32 DMA has no straggler (~1.135us/desc uniform, v1 trace).
  - fp16 groups are cheap on compute (PE pick ~6.5us, DVE 8.4, ACT 8.4 per
    8-row-tile group); f32 groups cost DVE 12.2 (STT pick) / ACT 11.6 (all
    solo EXP) but relieve engine 15.
  Mixing ~11 f32 groups among 30 balances: DMA15 ~325us, DVE ~308, ACT ~306,
  PE ~137, other DMA ~292 -> all under the old 368 floor.

fp16 group: SWDGE cast slab; one-hot mask16 = (iota16 == class) via
  fast-mode tensor_scalar; 8 chunked PE matmuls L16^T @ mask16 accumulated
  into one [125,125] PSUM bank across the whole kernel (trace = picked sum);
  4 solo EXP (ACT accum) + 1 batched EXP + 4 DVE reduces.
f32 group: HWDGE slab; v1 pick = scalar_tensor_tensor(iota32, class, L32)
  with accum (no fast mode, 1.41us/tile); all 8 tiles solo EXP with accum.
Box loss on DVE both ways; GpSimd only does SWDGE descr gen + iota consts.
Epilogue: logZ=Ln(sumexp) fused-sum; partial = logz_sum + 0.0625*loc_sum
  - picked(STT accum) - trace(PSUM diag); host sums partials / B.

Walrus quirks handled (from v1): single sync-wait per instruction
(multiwait split pass), no custom-DVE ISA ops, no TensorScalarPtr.
"""

import json as _json

import numpy as np

import concourse.bass as bass
import concourse.tile as tile
from concourse import mybir
import concourse.bass_utils as _bass_utils
import concourse.bass2jax as _bass2jax
from concourse.bass_utils import run_bass_kernel_spmd

P = 128
B = 262144
C = 1004
NCLS = 1000
NCORES = 8
R = B // NCORES       # 32768 rows per core
T = R // P            # 256 row-tiles per core
CH = 125              # matmul chunk width
NCH = 8               # chunks per row-tile

F32 = mybir.dt.float32
F16 = mybir.dt.float16
ALU = mybir.AluOpType
ACTF = mybir.ActivationFunctionType


# --------------------------------------------------------------------------
# BIR post-pass: this image's walrus supports only one sync-wait per
# instruction; split extras onto NoOp carriers placed just before.
# --------------------------------------------------------------------------
def _split_multiwait_bir(bir_json: bytes) -> bytes:
    d = _json.loads(bir_json)
    changed = False
    for fn in d.get("functions", []):
        for blk in fn.get("blocks", []):
            insts = blk.get("instructions", [])
            out = []
            for ins in insts:
                si = ins.get("sync_info") or {}
                waits = si.get("on_wait") or []
                if len(waits) > 1:
                    changed = True
                    for i, w in enumerate(waits[:-1]):
                        out.append(
                            {
                                "debug": ins.get("debug", 0),
                                "engine": ins["engine"],
                                "ins": [],
                                "name": f"{ins['name']}-wsplit{i}",
                                "opcode": "NoOp",
                                "outs": [],
                                "sync_info": {"on_update": [], "on_wait": [w]},
                            }
                        )
                    ins["sync_info"]["on_wait"] = [waits[-1]]
                out.append(ins)
            blk["instructions"] = out
    if not changed:
        return bir_json
    return _json.dumps(d).encode()


_orig_compile_bir_kernel = _bass_utils.compile_bir_kernel


def _compile_bir_kernel_fixed(bir_json, tmpdir, neff_name="file.neff"):
    if isinstance(bir_json, str):
        bir_json = bir_json.encode()
    return _orig_compile_bir_kernel(_split_multiwait_bir(bir_json), tmpdir, neff_name)


if _bass_utils.compile_bir_kernel is not _compile_bir_kernel_fixed:
    _bass_utils.compile_bir_kernel = _compile_bir_kernel_fixed
    _bass2jax.compile_bir_kernel = _compile_bir_kernel_fixed


# --------------------------------------------------------------------------
# group schedule: (size, kind, n_solo); kind "16" = SWDGE fp16 + PE pick,
# kind "32" = HWDGE f32 + STT pick (all solo EXP)
# --------------------------------------------------------------------------
N_F32_FULL = 11


def _schedule():
    groups = []
    for gs in (1, 1, 2, 4):                    # head: fp16, all-solo EXP
        groups.append((gs, "16", gs))
    # 30 full groups, N_F32_FULL of them f32, spread evenly
    for i in range(30):
        if ((i + 1) * N_F32_FULL) // 30 > (i * N_F32_FULL) // 30:
            groups.append((8, "32", 8))
        else:
            groups.append((8, "16", 4))
    for gs in (4, 2, 1, 1):                    # tail: fp16, split drain
        groups.append((gs, "16", gs // 2))
    assert sum(g[0] for g in groups) == T, sum(g[0] for g in groups)
    return groups


# --------------------------------------------------------------------------
# kernel build
# --------------------------------------------------------------------------
def build():
    nc = bass.Bass()
    x = nc.dram_tensor("x", [R, C], F32, kind="ExternalInput")
    t = nc.dram_tensor("t", [R, 5], F32, kind="ExternalInput")
    out = nc.dram_tensor("partial", [P, 1], F32, kind="ExternalOutput")

    xv = x[:].rearrange("(p n) c -> p n c", p=P)   # [128, 256, 1004]
    tv = t[:].rearrange("(p n) f -> p n f", p=P)   # [128, 256, 5]

    groups = _schedule()
    n_groups = len(groups)
    n_tiles32 = sum(g[0] for g in groups if g[1] == "32")

    with tile.TileContext(nc) as tc:
        with (
            tc.tile_pool(name="d16", bufs=3) as d16_pool,
            tc.tile_pool(name="d32", bufs=3) as d32_pool,
            tc.tile_pool(name="mask", bufs=6) as mask_pool,
            tc.tile_pool(name="exps", bufs=2) as exps_pool,
            tc.tile_pool(name="expb", bufs=3) as expb_pool,
            tc.tile_pool(name="pick", bufs=2) as pick_pool,
            tc.tile_pool(name="scr", bufs=3) as scr_pool,
            tc.tile_pool(name="acc", bufs=1) as acc_pool,
            tc.tile_pool(name="ps", bufs=1, space="PSUM") as psum_pool,
        ):
            # resident constants / accumulators
            tgt = acc_pool.tile([P, T, 5], F32)
            nc.sync.dma_start(out=tgt, in_=tv)
            iota16 = acc_pool.tile([P, NCLS], F16)
            nc.gpsimd.iota(iota16, pattern=[[1, NCLS]], base=0,
                           channel_multiplier=0,
                           allow_small_or_imprecise_dtypes=True)
            iota32 = acc_pool.tile([P, NCLS], F32)
            nc.gpsimd.iota(iota32, pattern=[[1, NCLS]], base=0,
                           channel_multiplier=0,
                           allow_small_or_imprecise_dtypes=True)
            dio = acc_pool.tile([CH, CH], F16)
            nc.gpsimd.iota(dio, pattern=[[1, CH]], base=0,
                           channel_multiplier=-1,
                           allow_small_or_imprecise_dtypes=True)
            diagm = acc_pool.tile([CH, CH], F16)
            nc.vector.tensor_scalar(diagm, dio, 0.0, None, op0=ALU.is_equal)

            sumexp_all = acc_pool.tile([P, T], F32)
            picked32 = acc_pool.tile([P, max(n_tiles32, 1)], F32)
            loc_all = acc_pool.tile([P, n_groups], F32)
            psum = psum_pool.tile([CH, CH], F32)

            nmm = (T - n_tiles32) * NCH
            k_mm = 0
            idx32 = 0
            t0 = 0
            for grp, (gs, kind, n_solo) in enumerate(groups):
                if kind == "16":
                    slab = d16_pool.tile([P, gs, C], F16, tag="slab16")
                    nc.gpsimd.dma_start(out=slab, in_=xv[:, t0 : t0 + gs, :])
                else:
                    slab = d32_pool.tile([P, gs, C], F32, tag="slab32")
                    nc.sync.dma_start(out=slab, in_=xv[:, t0 : t0 + gs, :])

                # ---- box loss terms on DVE ----
                e4 = scr_pool.tile([P, 2, gs, 2], F32, tag="e4")
                u2 = scr_pool.tile([P, gs, 2], F32, tag="u2")
                t01 = tgt[:, t0 : t0 + gs, 0:2]
                t23 = tgt[:, t0 : t0 + gs, 2:4]
                bp01 = slab[:, :, 0:2]
                bp23 = slab[:, :, 2:4]
                nc.vector.tensor_tensor(u2, t01, t23, ALU.add)
                nc.vector.tensor_tensor(u2, u2, bp01, ALU.subtract)
                nc.vector.tensor_tensor(e4[:, 0], u2, bp01, ALU.subtract)
                nc.vector.tensor_tensor(u2, t23, t01, ALU.subtract)
                nc.vector.tensor_tensor(u2, u2, bp23, ALU.subtract)
                nc.vector.tensor_tensor(e4[:, 1], u2, u2, ALU.add)
                nc.vector.tensor_tensor(e4, e4, e4, ALU.mult)
                nc.vector.tensor_reduce(
                    out=loc_all[:, grp : grp + 1], in_=e4,
                    axis=mybir.AxisListType.XYZ, op=ALU.add,
                )

                # ---- pick ----
                if kind == "16":
                    for g in range(gs):
                        tt = t0 + g
                        mask = mask_pool.tile([P, NCLS], F16, tag="mask")
                        nc.vector.tensor_scalar(
                            mask, iota16, tgt[:, tt, 4:5], None, op0=ALU.is_equal
                        )
                        for c in range(NCH):
                            nc.tensor.matmul(
                                psum,
                                slab[:, g, 4 + c * CH : 4 + (c + 1) * CH],
                                mask[:, c * CH : (c + 1) * CH],
                                start=(k_mm == 0),
                                stop=(k_mm == nmm - 1),
                            )
                            k_mm += 1
                else:
                    for g in range(gs):
                        tt = t0 + g
                        pick_scr = pick_pool.tile([P, NCLS], F32, tag="pick_scr")
                        nc.vector.scalar_tensor_tensor(
                            pick_scr,
                            iota32,
                            tgt[:, tt, 4:5],
                            slab[:, g, 4:C],
                            ALU.is_equal,
                            ALU.mult,
                            accum_out=picked32[:, idx32 : idx32 + 1],
                        )
                        idx32 += 1

                # ---- sumexp: n_solo solo (ACT accum), rest batched+DVE ----
                for g in range(n_solo):
                    tt = t0 + g
                    exp_s = exps_pool.tile([P, NCLS], F16, tag="exp_s")
                    nc.scalar.activation(
                        out=exp_s, in_=slab[:, g, 4:C], func=ACTF.Exp,
                        accum_out=sumexp_all[:, tt : tt + 1],
                    )
                if n_solo < gs:
                    nb = gs - n_solo
                    exp_b = expb_pool.tile([P, nb, C], F16, tag="exp_b")
                    nc.scalar.activation(
                        out=exp_b, in_=slab[:, n_solo:gs, :], func=ACTF.Exp
                    )
                    for j in range(nb):
                        tt = t0 + n_solo + j
                        nc.vector.tensor_reduce(
                            out=sumexp_all[:, tt : tt + 1],
                            in_=exp_b[:, j, 4:C],
                            axis=mybir.AxisListType.X, op=ALU.add,
                        )
                t0 += gs

            # ---- epilogue ----
            logz_scr = acc_pool.tile([P, T], F32)
            logz_sum = acc_pool.tile([P, 1], F32)
            nc.scalar.activation(
                out=logz_scr, in_=sumexp_all, func=ACTF.Ln, accum_out=logz_sum
            )
            loc_sum = acc_pool.tile([P, 1], F32)
            nc.vector.tensor_reduce(
                out=loc_sum, in_=loc_all, axis=mybir.AxisListType.X, op=ALU.add
            )
            pick32_sum = acc_pool.tile([P, 1], F32)
            nc.vector.tensor_reduce(
                out=pick32_sum, in_=picked32, axis=mybir.AxisListType.X, op=ALU.add
            )
            # trace of the PSUM bank (fp16-group picks)
            dtmp = acc_pool.tile([CH, CH], F32)
            nc.vector.tensor_tensor(dtmp, psum, diagm, ALU.mult)
            pick16 = acc_pool.tile([CH, 1], F32)
            nc.vector.tensor_reduce(out=pick16, in_=dtmp,
                                    axis=mybir.AxisListType.X, op=ALU.add)

            s = acc_pool.tile([P, 1], F32)
            # loc_all holds (2*err)^2 sums -> mean over 4 comps with the
            # doubling correction is 0.25 * 0.25 = 0.0625
            nc.vector.scalar_tensor_tensor(
                s, loc_sum, 0.0625, logz_sum, ALU.mult, ALU.add
            )
            nc.vector.tensor_tensor(s, s, pick32_sum, ALU.subtract)
            nc.vector.tensor_tensor(s[0:CH], s[0:CH], pick16, ALU.subtract)
            nc.sync.dma_start(out=out[:], in_=s)

    return nc


def _run(output, target, **spmd_kwargs):
    output = np.ascontiguousarray(np.asarray(output, dtype=np.float32))
    target = np.ascontiguousarray(np.asarray(target, dtype=np.float32))
    assert output.shape == (B, C), output.shape
    assert target.shape == (B, 5), target.shape
    nc = build()
    in_maps = [
        {
            "x": output[i * R : (i + 1) * R],
            "t": target[i * R : (i + 1) * R],
        }
        for i in range(NCORES)
    ]
    res = run_bass_kernel_spmd(nc, in_maps, core_ids=list(range(NCORES)), **spmd_kwargs)
    total = 0.0
    for r in res.results:
        total += r["partial"].astype(np.float64).sum()
    return np.float32(total / B), res


def kernel(output, target):
    val, _ = _run(output, target)
    return np.asarray(val, dtype=np.float32)


def kernel_profiled(output, target, **kw):
    """Returns (scalar, BassKernelResults) with trace for perf analysis."""
    return _run(output, target, trace=True, **kw)
